# revision 7
# baseline (speedup 1.0000x reference)
"""RGCN message-scoring kernel for Trainium2 (8 NeuronCores, SPMD).

Strategy (sharding_hint: partition graphs across devices):
- 250 graphs of exactly 200 nodes / 3200 within-graph edges are split across
  8 cores ([32,32,31,...,31], padded with zero "dummy" graphs to 32 each).
- Host does index-only preprocessing: per-graph normalized adjacency
  operator B^T[src, (rel,dst)] (counts -> 1/cnt edge weights, bf16), plus
  layout rearrangement of the dense inputs. All FP model math (both RGCN
  layers, message MLP, scoring) runs on device.
- Device (per graph): T1 = x^T @ B (PE), transform via W1/root1 (PE),
  ReLU (ACT), then layer 2 is collapsed against the per-graph message
  vector: psi = h1 @ [W2_r m | root2 m] and scores = sum_r psi_r^T B_r
  (PE, relation-major key layout), so layer 2 never materializes node
  embeddings.
- Output [250, 200] is a per-graph reshape of the per-node scores (each
  graph has exactly max_nodes nodes, so no -inf padding survives).
"""

import numpy as np

NG_FULL = 250       # total graphs
NPG = 200           # nodes per graph
EPG = 3200          # edges per graph
R = 8               # relations
F = 128             # feature/embedding width
G = 32              # graphs per core (padded)
NCORES = 8
KEYS = R * NPG      # 1600, relation-major: k = r*200 + dst_local

_COMPILED = {}


def _bf16(a):
    import ml_dtypes
    return np.ascontiguousarray(np.asarray(a, np.float32)).astype(ml_dtypes.bfloat16)


def _build_program():
    import concourse.bacc as bacc
    import concourse.mybir as mybir
    from concourse import tile

    dt = mybir.dt
    AF = mybir.ActivationFunctionType

    nc = bacc.Bacc("TRN2", target_bir_lowering=False, debug=False)

    # ---- dram io ----
    XL = nc.dram_tensor("XL", [128, G * 2 * 128], dt.bfloat16, kind="ExternalInput")
    XT = nc.dram_tensor("XT", [128, G * NPG], dt.bfloat16, kind="ExternalInput")
    BM = nc.dram_tensor("BM", [G * NPG, KEYS], dt.bfloat16, kind="ExternalInput")
    W1L = nc.dram_tensor("W1L", [128, R * 128], dt.bfloat16, kind="ExternalInput")
    ROOT1 = nc.dram_tensor("ROOT1", [128, 128], dt.bfloat16, kind="ExternalInput")
    B1 = nc.dram_tensor("B1", [128, 1], dt.float32, kind="ExternalInput")
    W2T = nc.dram_tensor("W2T", [128, R * 128], dt.bfloat16, kind="ExternalInput")
    ROOT2T = nc.dram_tensor("ROOT2T", [128, 128], dt.bfloat16, kind="ExternalInput")
    B2 = nc.dram_tensor("B2", [128, 1], dt.bfloat16, kind="ExternalInput")
    EMBL = nc.dram_tensor("EMBL", [128, 8 * 128], dt.bfloat16, kind="ExternalInput")
    SEL = nc.dram_tensor("SEL", [128, 8 * G], dt.bfloat16, kind="ExternalInput")
    CONT = nc.dram_tensor("CONT", [1, G], dt.bfloat16, kind="ExternalInput")
    CONTW = nc.dram_tensor("CONTW", [1, 128], dt.bfloat16, kind="ExternalInput")
    CONTB = nc.dram_tensor("CONTB", [128, 1], dt.float32, kind="ExternalInput")
    MSGW = nc.dram_tensor("MSGW", [128, 2 * 128], dt.bfloat16, kind="ExternalInput")
    MSGB = nc.dram_tensor("MSGB", [128, 1], dt.float32, kind="ExternalInput")
    OUT = nc.dram_tensor("OUT", [1, G * NPG], dt.float32, kind="ExternalOutput")

    with tile.TileContext(nc) as tc:
        with (
            tc.tile_pool(name="const", bufs=1) as const,
            tc.tile_pool(name="bpool", bufs=4) as bpool,
            tc.tile_pool(name="t1pool", bufs=2) as t1pool,
            tc.tile_pool(name="hpool", bufs=2) as hpool,
        ):
            # ---- load resident constants ----
            xl = const.tile([128, G * 2 * 128], dt.bfloat16)
            xt = const.tile([128, G * NPG], dt.bfloat16)
            w1l = const.tile([128, R * 128], dt.bfloat16)
            root1 = const.tile([128, 128], dt.bfloat16)
            b1 = const.tile([128, 1], dt.float32)
            w2t = const.tile([128, R * 128], dt.bfloat16)
            root2t = const.tile([128, 128], dt.bfloat16)
            b2 = const.tile([128, 1], dt.bfloat16)
            nc.sync.dma_start(xl[:], XL.ap()[:])
            nc.sync.dma_start(xt[:], XT.ap()[:])
            nc.sync.dma_start(w1l[:], W1L.ap()[:])
            nc.sync.dma_start(root1[:], ROOT1.ap()[:])
            nc.sync.dma_start(b1[:], B1.ap()[:])
            nc.sync.dma_start(w2t[:], W2T.ap()[:])
            nc.sync.dma_start(root2t[:], ROOT2T.ap()[:])
            nc.sync.dma_start(b2[:], B2.ap()[:])

            # w2mB layout: [128, 9*G], column n*G + g holds (W2_n m_g) for
            # n<8 and (root2 m_g) for n==8.
            w2mB = const.tile([128, 9 * G], dt.bfloat16)
            b2m = const.tile([1, G], dt.float32)
            out_sb = const.tile([1, G * NPG], dt.float32)

            # ---- message stage (own psum scope, closed before graph loop) ----
            with (
                tc.tile_pool(name="msg", bufs=1) as msg,
                tc.tile_pool(name="psm", bufs=2, space="PSUM") as psm,
            ):
                embl = msg.tile([128, 8 * 128], dt.bfloat16)
                sel = msg.tile([128, 8 * G], dt.bfloat16)
                cont = msg.tile([1, G], dt.bfloat16)
                contw = msg.tile([1, 128], dt.bfloat16)
                contb = msg.tile([128, 1], dt.float32)
                msgw = msg.tile([128, 2 * 128], dt.bfloat16)
                msgb = msg.tile([128, 1], dt.float32)
                nc.sync.dma_start(embl[:], EMBL.ap()[:])
                nc.sync.dma_start(sel[:], SEL.ap()[:])
                nc.sync.dma_start(cont[:], CONT.ap()[:])
                nc.sync.dma_start(contw[:], CONTW.ap()[:])
                nc.sync.dma_start(contb[:], CONTB.ap()[:])
                nc.sync.dma_start(msgw[:], MSGW.ap()[:])
                nc.sync.dma_start(msgb[:], MSGB.ap()[:])

                # disc_embT [128f, G] = sum_c EMBL_c^T @ SEL_c
                ps_d = psm.tile([128, G], dt.float32)
                for c in range(8):
                    nc.tensor.matmul(
                        ps_d[:], embl[:, c * 128:(c + 1) * 128], sel[:, c * G:(c + 1) * G],
                        start=(c == 0), stop=(c == 7))
                discT = msg.tile([128, G], dt.bfloat16)
                nc.vector.tensor_copy(discT[:], ps_d[:])

                # cont_embT [128, G] = relu(cont_w^T cont + cont_b)
                ps_c = psm.tile([128, G], dt.float32)
                nc.tensor.matmul(ps_c[:], contw[:], cont[:], start=True, stop=True)
                contT = msg.tile([128, G], dt.bfloat16)
                nc.scalar.activation(contT[:], ps_c[:], AF.Relu, bias=contb[:])

                # mT [128, G] = relu(msg_w^T [disc; cont] + msg_b)
                ps_m = psm.tile([128, G], dt.float32)
                nc.tensor.matmul(ps_m[:], msgw[:, 0:128], discT[:], start=True, stop=False)
                nc.tensor.matmul(ps_m[:], msgw[:, 128:256], contT[:], start=False, stop=True)
                mT = msg.tile([128, G], dt.bfloat16)
                nc.scalar.activation(mT[:], ps_m[:], AF.Relu, bias=msgb[:])

                # w2m[r] = W2_r m ; rootm = root2 m ; b2m = b2 . m
                for r in range(R):
                    ps_w = psm.tile([128, G], dt.float32, tag="psw")
                    nc.tensor.matmul(ps_w[:], w2t[:, r * 128:(r + 1) * 128], mT[:],
                                     start=True, stop=True)
                    nc.vector.tensor_copy(w2mB[:, r * G:(r + 1) * G], ps_w[:])
                ps_r = psm.tile([128, G], dt.float32, tag="psw")
                nc.tensor.matmul(ps_r[:], root2t[:], mT[:], start=True, stop=True)
                nc.vector.tensor_copy(w2mB[:, 8 * G:9 * G], ps_r[:])
                ps_b = psm.tile([1, G], dt.float32, tag="psw")
                nc.tensor.matmul(ps_b[:], b2[:], mT[:], start=True, stop=True)
                nc.vector.tensor_copy(b2m[:], ps_b[:])

            # ---- per-graph pipeline ----
            w2mB3 = w2mB[:].rearrange("p (n g) -> p n g", g=G)
            with (
                tc.tile_pool(name="pst1", bufs=4, space="PSUM") as pst1,
                tc.tile_pool(name="psa", bufs=2, space="PSUM") as psa,
                tc.tile_pool(name="psp", bufs=1, space="PSUM") as psp,
                tc.tile_pool(name="pss", bufs=1, space="PSUM") as pss,
            ):
                for g in range(G):
                    bc0 = bpool.tile([128, KEYS], dt.bfloat16, tag="bc")
                    bc1 = bpool.tile([128, KEYS], dt.bfloat16, tag="bc")
                    nc.sync.dma_start(bc0[:], BM.ap()[g * NPG:g * NPG + 128, :])
                    # rows 72.. of chunk 1 are not loaded; zero them so the
                    # zero-padded lhsT rows never multiply NaN/Inf bit patterns
                    nc.gpsimd.memset(bc1[64:128, :], 0.0)
                    nc.sync.dma_start(bc1[:72, :], BM.ap()[g * NPG + 128:(g + 1) * NPG, :])

                    # T1^T [128f, 1600k] = sum_c XL_gc^T @ B_gc, in 4 key-blocks
                    t1sb = t1pool.tile([128, KEYS], dt.bfloat16)
                    for q in range(4):
                        t1p = pst1.tile([128, 400], dt.float32)
                        for c, bc in enumerate((bc0, bc1)):
                            nc.tensor.matmul(
                                t1p[:],
                                xl[:, (g * 2 + c) * 128:(g * 2 + c + 1) * 128],
                                bc[:, q * 400:(q + 1) * 400],
                                start=(c == 0), stop=(c == 1))
                        if q % 2 == 0:
                            nc.vector.tensor_copy(t1sb[:, q * 400:(q + 1) * 400], t1p[:])
                        else:
                            nc.scalar.activation(t1sb[:, q * 400:(q + 1) * 400], t1p[:],
                                                 AF.Copy)

                    # transform: agg1^T [128, 200] = sum_r W1_r^T T1_r + root1^T x^T
                    a1 = psa.tile([128, NPG], dt.float32)
                    for r in range(R):
                        nc.tensor.matmul(a1[:], w1l[:, r * 128:(r + 1) * 128],
                                         t1sb[:, r * NPG:(r + 1) * NPG],
                                         start=(r == 0), stop=False)
                    nc.tensor.matmul(a1[:], root1[:], xt[:, g * NPG:(g + 1) * NPG],
                                     start=False, stop=True)
                    h1 = hpool.tile([128, NPG], dt.bfloat16)
                    nc.scalar.activation(h1[:], a1[:], AF.Relu, bias=b1[:])

                    # psi [node, 9] per node-chunk
                    psi_p = psp.tile([128, 18], dt.float32)
                    nc.tensor.matmul(psi_p[:, 0:9], h1[:, 0:128],
                                     w2mB3[:, :, g], start=True, stop=True)
                    nc.tensor.matmul(psi_p[:72, 9:18], h1[:, 128:200],
                                     w2mB3[:, :, g], start=True, stop=True)
                    psi = hpool.tile([128, 18], dt.bfloat16, tag="psi")
                    nc.vector.memset(psi[:, 9:18], 0.0)
                    nc.vector.tensor_copy(psi[:, 0:9], psi_p[:, 0:9])
                    nc.vector.tensor_copy(psi[:72, 9:18], psi_p[:72, 9:18])

                    # scores [1, 200] = sum_{c,r} psi_cr^T B_cr + (root2 m)^T h1
                    sc = pss.tile([1, NPG], dt.float32)
                    first = True
                    for c, bc in enumerate((bc0, bc1)):
                        for r in range(R):
                            nc.tensor.matmul(sc[:], psi[:, c * 9 + r: c * 9 + r + 1],
                                             bc[:, r * NPG:(r + 1) * NPG],
                                             start=first, stop=False)
                            first = False
                    nc.tensor.matmul(sc[:], w2mB[:, 8 * G + g: 8 * G + g + 1], h1[:],
                                     start=False, stop=True)
                    nc.scalar.activation(out_sb[0:1, g * NPG:(g + 1) * NPG], sc[:], AF.Identity,
                                         bias=b2m[0:1, g:g + 1])

            nc.sync.dma_start(OUT.ap()[:], out_sb[:])

    nc.compile()
    return nc


def _np_reference(message, x, edge_index, edge_type, batch, max_nodes,
                  W1, root1, b1, W2, root2, b2,
                  embed_table, cont_w, cont_b, msg_w, msg_b):
    """Pure-numpy fallback for inputs that violate the regular-structure
    assumptions (ragged batches or cross-graph edges)."""
    n_nodes, n_rel, n_graphs = x.shape[0], W1.shape[0], message.shape[0]
    src, dst = edge_index[0], edge_index[1]

    def conv(h, W, root, b):
        hW = np.einsum('nf,rfo->nro', h, W)
        m = hW[src, edge_type]
        key_dr = dst * n_rel + edge_type
        cnt = np.zeros(n_nodes * n_rel, h.dtype)
        np.add.at(cnt, key_dr, 1.0)
        nrm = 1.0 / np.maximum(cnt[key_dr], 1.0)
        agg = np.zeros((n_nodes, W.shape[2]), h.dtype)
        np.add.at(agg, dst, m * nrm[:, None])
        return agg + h @ root + b

    h = np.maximum(conv(x, W1, root1, b1), 0)
    node_emb = conv(h, W2, root2, b2)
    disc = embed_table[message[:, 0].astype(np.int32)]
    cont = np.maximum(message[:, 1:2].astype(np.float32) @ cont_w + cont_b, 0)
    mrep = np.maximum(np.concatenate([disc, cont], 1) @ msg_w + msg_b, 0)
    scores = (node_emb * mrep[batch]).sum(1)
    cnts = np.bincount(batch, minlength=n_graphs)
    start = np.cumsum(cnts) - cnts
    pos = np.arange(n_nodes) - start[batch]
    logits = np.full((n_graphs, int(max_nodes)), -np.inf, np.float32)
    ok = pos < int(max_nodes)  # jax .at[].set drops OOB indices; match that
    logits[batch[ok], pos[ok]] = scores.astype(np.float32)[ok]
    return logits


def kernel(**inputs):
    message = np.asarray(inputs["message"], np.float32)
    x = np.asarray(inputs["x"], np.float32)
    edge_index = np.asarray(inputs["edge_index"])
    edge_type = np.asarray(inputs["edge_type"])
    batch = np.asarray(inputs["batch"])
    max_nodes = int(np.asarray(inputs["max_nodes"]))
    W1 = np.asarray(inputs["W1"], np.float32)
    root1 = np.asarray(inputs["root1"], np.float32)
    b1 = np.asarray(inputs["b1"], np.float32)
    W2 = np.asarray(inputs["W2"], np.float32)
    root2 = np.asarray(inputs["root2"], np.float32)
    b2 = np.asarray(inputs["b2"], np.float32)
    embed_table = np.asarray(inputs["embed_table"], np.float32)
    cont_w = np.asarray(inputs["cont_w"], np.float32)
    cont_b = np.asarray(inputs["cont_b"], np.float32)
    msg_w = np.asarray(inputs["msg_w"], np.float32)
    msg_b = np.asarray(inputs["msg_b"], np.float32)

    ng = message.shape[0]
    src, dst = edge_index[0].astype(np.int64), edge_index[1].astype(np.int64)
    et = edge_type.astype(np.int64)

    regular = (
        ng == NG_FULL
        and x.shape == (NG_FULL * NPG, F)
        and max_nodes == NPG
        and W1.shape == (R, F, F)
        and src.shape[0] == NG_FULL * EPG
        and embed_table.shape == (1000, F)
        and np.array_equal(batch, np.repeat(np.arange(ng), NPG))
        and np.array_equal(src // NPG, np.repeat(np.arange(ng), EPG))
        and np.array_equal(dst // NPG, np.repeat(np.arange(ng), EPG))
        and et.min() >= 0 and et.max() < R
        and message[:, 0].min() >= 0 and message[:, 0].max() < 1000
    )
    if not regular:
        return _np_reference(**inputs)

    # ---- host index preprocessing: normalized per-graph operator B ----
    eg = dst // NPG
    dst_l = dst % NPG
    src_l = src % NPG
    key = et * NPG + dst_l                       # relation-major local key
    gk = eg * KEYS + key
    cnt = np.bincount(gk, minlength=NG_FULL * KEYS).astype(np.float32)
    norm = 1.0 / np.maximum(cnt, 1.0)
    B = np.zeros((NG_FULL * NPG, KEYS), np.float32)
    np.add.at(B, (eg * NPG + src_l, key), norm[gk])
    B = _bf16(B)

    counts = [32, 32, 31, 31, 31, 31, 31, 31]
    starts = np.concatenate([[0], np.cumsum(counts)])[:-1]
    tok = message[:, 0].astype(np.int64)
    contv = message[:, 1]

    xb = _bf16(x)
    in_maps = []
    for c in range(NCORES):
        g0, gc = int(starts[c]), counts[c]
        # XL: [p, (g,2,f)] node-chunked lhsT layout, zero-padded to 32 graphs
        xl = np.zeros((G, 2, 128, F), xb.dtype)
        xg = xb[g0 * NPG:(g0 + gc) * NPG].reshape(gc, NPG, F)
        xl[:gc, 0] = xg[:, 0:128]
        xl[:gc, 1, 0:72] = xg[:, 128:200]
        xl = np.ascontiguousarray(xl.transpose(2, 0, 1, 3).reshape(128, G * 2 * F))
        # XT: x^T
        xt = np.zeros((128, G * NPG), xb.dtype)
        xt[:, :gc * NPG] = xg.reshape(gc * NPG, F).T
        # BM
        bm = np.zeros((G * NPG, KEYS), B.dtype)
        bm[:gc * NPG] = B[g0 * NPG:(g0 + gc) * NPG]
        # message-side inputs
        selv = np.zeros((1024, G), np.float32)
        selv[tok[g0:g0 + gc], np.arange(gc)] = 1.0
        sel = _bf16(selv.reshape(8, 128, G).transpose(1, 0, 2).reshape(128, 8 * G))
        cont_row = np.zeros((1, G), np.float32)
        cont_row[0, :gc] = contv[g0:g0 + gc]
        embl = np.zeros((1024, F), np.float32)
        embl[:1000] = embed_table
        embl = _bf16(embl.reshape(8, 128, F).transpose(1, 0, 2).reshape(128, 8 * F))

        in_maps.append({
            "XL": xl, "XT": xt, "BM": bm,
            "W1L": _bf16(W1.transpose(1, 0, 2).reshape(128, R * 128)),
            "ROOT1": _bf16(root1),
            "B1": b1.reshape(128, 1).astype(np.float32),
            "W2T": _bf16(W2.transpose(2, 0, 1).reshape(128, R * 128)),
            "ROOT2T": _bf16(root2.T),
            "B2": _bf16(b2.reshape(128, 1)),
            "EMBL": embl, "SEL": sel,
            "CONT": _bf16(cont_row),
            "CONTW": _bf16(cont_w),
            "CONTB": cont_b.reshape(128, 1).astype(np.float32),
            "MSGW": _bf16(msg_w.reshape(2, 128, 128).transpose(1, 0, 2).reshape(128, 256)),
            "MSGB": msg_b.reshape(128, 1).astype(np.float32),
        })

    from concourse.bass_utils import run_bass_kernel_spmd
    if "nc" not in _COMPILED:
        _COMPILED["nc"] = _build_program()
    global _LAST_IN_MAPS
    _LAST_IN_MAPS = in_maps
    res = run_bass_kernel_spmd(_COMPILED["nc"], in_maps, core_ids=list(range(NCORES)))

    out = np.empty((NG_FULL, NPG), np.float32)
    for c in range(NCORES):
        g0, gc = int(starts[c]), counts[c]
        out[g0:g0 + gc] = res.results[c]["OUT"].reshape(G, NPG)[:gc]
    return out
